# revision 1
# baseline (speedup 1.0000x reference)
"""GAT layer (nn_GATLayer) on 8 Trainium2 NeuronCores via Bass/Tile.

Strategy (dst-partitioned, softmax fully local per core):
  - Core k owns dst nodes [k*6250, (k+1)*6250). All in-edges of those nodes
    are processed on core k, so segment-softmax needs no cross-core reduction.
  - Phase A (replicated): z = h @ W.T for ALL nodes, with the attention
    projections folded into the weight matrix: rhs = [W.T | W.T@A1blk | W.T@A2blk]
    giving per-node rows [z(64) | as(4) | ad(4)] in one matmul (h, W in bf16;
    f32 PSUM accumulation). Rows packed as [z bf16 x64 | as f32 x4 | pad]
    (256B) in HBM table `zaug`.
  - Phase A0: same matmul over own nodes only -> `adpad` table [6272, 64] rows
    holding ad(4) per own node (256B rows).
  - Edge phase: dst nodes are bin-packed into 49 blocks of <=128 nodes,
    balancing per-block lo/hi edge counts (gather indices are int16, so src
    is split at 32768 into two streams over zaug views). Per block:
    dma_gather of zaug rows by src (<=1024 indices per call: SWDGE ring cap),
    dma_gather of adpad rows by local dst (hoisted into a pre-loop so its
    descriptor-gen and DMA overlap phase A). e = as_src + ad_dst; leaky_relu (max(e, 0.2e)); ex = exp(e)
    (no max subtraction needed: |e| <~ 1.5, softmax is shift-invariant).
    rhs = [ex*zs | ex]; per 128-edge chunk a one-hot selection matrix
    sel[p, n] = (pos[p] == n) (pos = the dst node's position in its block) is
    built on DVE and PSUM-accumulated: acc[n, :] += sel.T @ rhs -> [num|denom].
  - Flush: out = num / max(denom, 1e-12), stored per block (position-major);
    the host permutes rows back to node order.

Edges are assigned to (block, chunk-slot) wholly on the host; pad slots get
pos = -1000 so their sel column is all zero (contribute nothing).
"""

import math

import numpy as np

N_NODES = 50000
N_EDGES = 800000
IN_FEATS = 128
OUT_FEATS = 16
NUM_HEADS = 4
ALPHA = 0.2
HF = NUM_HEADS * OUT_FEATS  # 64

N_CORES = 8
P = 128
NODES_PER_CORE = N_NODES // N_CORES     # 6250
BLOCKS = 49                              # ceil(6250/128) blocks of 128 dst nodes
NODE_PAD = BLOCKS * P                    # 6272
SPLIT = 32768                            # int16 gather index limit
N_NODES_PAD = 50048                      # 391 * 128


def _build_host_data(h, src, dst, W, A):
    """All index/layout prep. Returns per-core input dicts + static dims."""
    src = np.asarray(src)
    dst = np.asarray(dst)
    W = np.asarray(W, dtype=np.float32)
    A = np.asarray(A, dtype=np.float32)
    h = np.asarray(h, dtype=np.float32)

    # folded weights: [W.T | W.T@A1blk | W.T@A2blk]  -> [128, 72]
    A1blk = np.zeros((HF, NUM_HEADS), dtype=np.float32)
    A2blk = np.zeros((HF, NUM_HEADS), dtype=np.float32)
    for hd in range(NUM_HEADS):
        A1blk[hd * OUT_FEATS:(hd + 1) * OUT_FEATS, hd] = A[hd, :OUT_FEATS]
        A2blk[hd * OUT_FEATS:(hd + 1) * OUT_FEATS, hd] = A[hd, OUT_FEATS:]
    WT = np.ascontiguousarray(W.T)                       # [128, 64]
    import ml_dtypes
    wcat = np.concatenate([WT, WT @ A1blk, WT @ A2blk], axis=1)  # [128, 72]
    wcat = wcat.astype(ml_dtypes.bfloat16)

    hT = np.zeros((P, N_NODES_PAD), dtype=ml_dtypes.bfloat16)    # [128, 50048]
    hT[:, :N_NODES] = h.T.astype(ml_dtypes.bfloat16)

    # per-core edge slots
    order = np.argsort(dst, kind="stable")
    dst_s = dst[order]
    src_s = src[order]
    core_begin = np.searchsorted(dst_s, np.arange(0, N_NODES + 1, NODES_PER_CORE))

    # Assign nodes to blocks by bin-packing lo/hi edge counts (balances the
    # per-block chunk budgets KLO/KHI so the global maxima carry little pad).
    per_core = []
    klo_max, khi_max = 1, 1
    for k in range(N_CORES):
        lo_e, hi_e = core_begin[k], core_begin[k + 1]
        cs = src_s[lo_e:hi_e]
        cd = dst_s[lo_e:hi_e] - k * NODES_PER_CORE      # local dst [0, 6250)
        is_lo = cs < SPLIT
        lo_deg = np.bincount(cd[is_lo], minlength=NODES_PER_CORE)
        hi_deg = np.bincount(cd[~is_lo], minlength=NODES_PER_CORE)
        tot_lo, tot_hi = lo_deg.sum(), hi_deg.sum()
        # greedy LPT on the dominant stream
        order_n = np.argsort(-(lo_deg + hi_deg), kind="stable")
        bl_lo = np.zeros(BLOCKS, dtype=np.int64)
        bl_hi = np.zeros(BLOCKS, dtype=np.int64)
        bl_cnt = np.zeros(BLOCKS, dtype=np.int64)
        node_block = np.empty(NODES_PER_CORE, dtype=np.int64)
        node_pos = np.empty(NODES_PER_CORE, dtype=np.int64)
        # vectorized-ish greedy: process nodes in order, pick argmin of score
        wl = 1.0 / max(tot_lo, 1)
        wh = 1.0 / max(tot_hi, 1)
        for n in order_n:
            score = np.maximum(bl_lo * wl, bl_hi * wh) + np.where(bl_cnt >= P, 1e9, 0.0)
            b = int(np.argmin(score))
            node_block[n] = b
            node_pos[n] = bl_cnt[b]
            bl_cnt[b] += 1
            bl_lo[b] += lo_deg[n]
            bl_hi[b] += hi_deg[n]
        klo_max = max(klo_max, math.ceil(bl_lo.max() / P))
        khi_max = max(khi_max, math.ceil(bl_hi.max() / P))
        # per-block edge lists
        eb = node_block[cd]
        blocks_lo = []
        blocks_hi = []
        for b in range(BLOCKS):
            ml = (eb == b) & is_lo
            mh = (eb == b) & ~is_lo
            blocks_lo.append((cs[ml], cd[ml]))
            blocks_hi.append((cs[mh], cd[mh]))
        per_core.append((blocks_lo, blocks_hi, node_block, node_pos))

    KLO, KHI = klo_max, khi_max
    K = KLO + KHI                 # chunks per block
    NLO, NHI = KLO * P, KHI * P   # slots per block per stream

    in_maps = []
    unpack_maps = []
    for k in range(N_CORES):
        blocks_lo, blocks_hi, node_block, node_pos = per_core[k]
        unpack_maps.append(node_block * P + node_pos)   # node -> outO row
        # slot tables
        gidx_lo = np.zeros((BLOCKS, NLO), dtype=np.int16)   # src (pads = 0, all valid)
        gidx_hi = np.zeros((BLOCKS, NHI), dtype=np.int16)
        aidx = np.zeros((BLOCKS, K * P), dtype=np.int16)    # local dst (pad 0)
        dstv = np.full((BLOCKS, K * P), -1000.0, dtype=np.float32)  # rel dst per slot

        for b in range(BLOCKS):
            sl, dl = blocks_lo[b]
            sh, dh = blocks_hi[b]
            nl, nh = len(sl), len(sh)
            gidx_lo[b, :nl] = sl
            gidx_lo[b, nl:] = 0
            gidx_hi[b, :nh] = sh - SPLIT
            gidx_hi[b, nh:] = 0
            aidx[b, :nl] = dl
            aidx[b, NLO:NLO + nh] = dh
            dstv[b, :nl] = node_pos[dl].astype(np.float32)
            dstv[b, NLO:NLO + nh] = node_pos[dh].astype(np.float32)

        def wrap16(vals):
            # stream position i -> idx tile [16, n/16] at [i%16, i//16]; rows
            # replicated to 128 partitions (interp reads a [128, n/16] view).
            n = vals.shape[-1]
            w = vals.reshape(vals.shape[0], n // 16, 16)      # [B, s, p16]
            w = np.transpose(w, (0, 2, 1))                     # [B, 16, s]
            out = np.tile(w, (1, 8, 1))                        # [B, 128, s]
            return np.ascontiguousarray(out)

        gl = wrap16(gidx_lo)     # [B, 128, NLO/16]
        gh = wrap16(gidx_hi)
        ga = wrap16(aidx)
        # dstv: slot (p, c) value at [p, b, c] ; stream i = c*128+p
        dv = dstv.reshape(BLOCKS, K, P).transpose(2, 0, 1)     # [128, B, K]

        hT_own = np.zeros((P, NODE_PAD), dtype=ml_dtypes.bfloat16)
        hT_own[:, :NODES_PER_CORE] = hT[:, k * NODES_PER_CORE:(k + 1) * NODES_PER_CORE]

        iota = np.ascontiguousarray(np.broadcast_to(
            np.arange(P, dtype=np.float32)[None, :], (P, P)))

        in_maps.append({
            "hT": hT,
            "hT_own": hT_own,
            "wcat": wcat,
            "gidx_lo": np.ascontiguousarray(gl.transpose(1, 0, 2).reshape(P, -1)),
            "gidx_hi": np.ascontiguousarray(gh.transpose(1, 0, 2).reshape(P, -1)),
            "aidx": np.ascontiguousarray(ga.transpose(1, 0, 2).reshape(P, -1)),
            "dstv": np.ascontiguousarray(dv.reshape(P, -1)),
            "iota": iota,
        })
    return in_maps, KLO, KHI, unpack_maps


def _build_program(KLO, KHI):
    import concourse.bacc as bacc
    import concourse.tile as tile
    import concourse.mybir as mybir

    K = KLO + KHI
    NLO, NHI = KLO * P, KHI * P
    NSLOT = K * P
    f32 = mybir.dt.float32
    bf16 = mybir.dt.bfloat16
    i16 = mybir.dt.int16

    import os as _os
    _simclean = _os.environ.get("SIM_CLEAN", "0") == "1"
    nc = bacc.Bacc("TRN2", target_bir_lowering=False, debug=False)

    hT = nc.dram_tensor("hT", [P, N_NODES_PAD], bf16, kind="ExternalInput")
    hT_own = nc.dram_tensor("hT_own", [P, NODE_PAD], bf16, kind="ExternalInput")
    wcat_d = nc.dram_tensor("wcat", [P, 72], bf16, kind="ExternalInput")
    gidx_lo = nc.dram_tensor("gidx_lo", [P, BLOCKS * NLO // 16], i16, kind="ExternalInput")
    gidx_hi = nc.dram_tensor("gidx_hi", [P, BLOCKS * NHI // 16], i16, kind="ExternalInput")
    aidx = nc.dram_tensor("aidx", [P, BLOCKS * NSLOT // 16], i16, kind="ExternalInput")
    dstv_d = nc.dram_tensor("dstv", [P, BLOCKS * K], f32, kind="ExternalInput")
    iota_d = nc.dram_tensor("iota", [P, P], f32, kind="ExternalInput")

    zaug = nc.dram_tensor("zaug", [N_NODES_PAD, 64], f32)  # bytes: [z bf16 x64 | as f32 x4 | pad]
    adpad = nc.dram_tensor("adpad", [NODE_PAD, 64], f32)        # [ad|pad]
    outO = nc.dram_tensor("outO", [NODE_PAD, HF], f32, kind="ExternalOutput")

    NCHUNK = N_NODES_PAD // P        # 391 node chunks govern phase A
    SC = 4                       # chunks per super-chunk

    with tile.TileContext(nc) as tc:
        with (
            tc.tile_pool(name="const", bufs=1) as cpool,
            tc.tile_pool(name="pa", bufs=8) as pa,
            tc.tile_pool(name="papsum", bufs=4, space="PSUM") as papsum,
            tc.tile_pool(name="edge", bufs=4) as ep,
            tc.tile_pool(name="sel", bufs=8) as selp,
            tc.tile_pool(name="accpsum", bufs=3, space="PSUM") as accp,
            tc.tile_pool(name="flush", bufs=3) as fp,
        ):
            wcat_t = cpool.tile([P, 72], bf16)
            nc.sync.dma_start(out=wcat_t[:], in_=wcat_d[:])
            iota_t = cpool.tile([P, P], f32)
            nc.sync.dma_start(out=iota_t[:], in_=iota_d[:])
            dstv_t = cpool.tile([P, BLOCKS * K], f32)
            nc.sync.dma_start(out=dstv_t[:], in_=dstv_d[:])
            gl_t = cpool.tile([P, BLOCKS * NLO // 16], i16)
            nc.sync.dma_start(out=gl_t[:], in_=gidx_lo[:])
            gh_t = cpool.tile([P, BLOCKS * NHI // 16], i16)
            nc.sync.dma_start(out=gh_t[:], in_=gidx_hi[:])
            ai_t = cpool.tile([P, BLOCKS * NSLOT // 16], i16)
            nc.sync.dma_start(out=ai_t[:], in_=aidx[:])

            # ---------------- Phase A0: adpad for own nodes ----------------
            n_sc_own = NODE_PAD // (SC * P)                # 12 (6144) + 1 partial (128)
            own_scs = [(s * SC * P, SC) for s in range(n_sc_own)]
            if NODE_PAD % (SC * P):
                own_scs.append((n_sc_own * SC * P, (NODE_PAD % (SC * P)) // P))
            for base, nsub in own_scs:
                hsl = pa.tile([P, SC * P], bf16, tag="hsl")
                nc.sync.dma_start(out=hsl[:, :nsub * P], in_=hT_own[:, base:base + nsub * P])
                zp = papsum.tile([P, SC, 72], f32, tag="zp")
                for j in range(nsub):
                    nc.tensor.matmul(
                        out=zp[:, j, :],
                        lhsT=hsl[:, j * P:(j + 1) * P],
                        rhs=wcat_t[:],
                        start=True, stop=True,
                    )
                adst = pa.tile([P, SC, 64], f32, tag="adst")
                nc.scalar.memzero(adst[:])  # adpad junk cols are gathered -> keep init
                nc.scalar.copy(out=adst[:, :nsub, 0:4], in_=zp[:, :nsub, 68:72])
                ad3 = adpad[:].rearrange("(s p) e -> p s e", p=P)
                nc.sync.dma_start(
                    out=ad3[:, base // P:base // P + nsub, :],
                    in_=adst[:, :nsub, :],
                )

            # ---------------- Phase A: zaug for all nodes ----------------
            SC2 = 2 * SC                                   # 8 chunks per load/store
            n_sc = NCHUNK // SC2
            all_scs = [(s * SC2 * P, SC2) for s in range(n_sc)]
            if NCHUNK % SC2:
                all_scs.append((n_sc * SC2 * P, NCHUNK % SC2))
            for base, nsub in all_scs:
                hsl = pa.tile([P, SC2 * P], bf16, tag="hsl")
                nc.sync.dma_start(out=hsl[:, :nsub * P], in_=hT[:, base:base + nsub * P])
                zst = pa.tile([P, SC2, 64], f32, tag="zst")
                if _simclean:
                    nc.scalar.memzero(zst[:])
                zbf = zst[:].bitcast(mybir.dt.bfloat16)
                for g0 in range(0, nsub, SC):
                    g1 = min(g0 + SC, nsub)
                    zp = papsum.tile([P, SC, 72], f32, tag="zp")
                    for j in range(g0, g1):
                        nc.tensor.matmul(
                            out=zp[:, j - g0, :],
                            lhsT=hsl[:, j * P:(j + 1) * P],
                            rhs=wcat_t[:],
                            start=True, stop=True,
                        )
                    nc.scalar.copy(out=zbf[:, g0:g1, 0:64], in_=zp[:, :g1 - g0, 0:64])
                    nc.scalar.copy(out=zst[:, g0:g1, 32:36], in_=zp[:, :g1 - g0, 64:68])
                z3 = zaug[:].rearrange("(s p) e -> p s e", p=P)
                nc.sync.dma_start(
                    out=z3[:, base // P:base // P + nsub, :],
                    in_=zst[:, :nsub, :],
                )

            # ---------------- Edge phase ----------------
            zaug_lo = zaug[0:SPLIT, :]
            zaug_hi = zaug[SPLIT:N_NODES_PAD, :]
            _n_iter = BLOCKS
            CMAX = 8  # chunks (1024 idxs) per dma_gather call — SWDGE ring cap
            # ad pre-loop: gather+compact dst attention terms for all blocks.
            # Runs early (only depends on adpad from phase A0) so its desc-gen
            # and DMA overlap phase A.
            adall = cpool.tile([P, BLOCKS, K, 4], f32)
            for b in range(_n_iter):
                adt = ep.tile([P, K, 64], f32, tag="adt")
                if True:
                    for c0 in range(0, K, CMAX):
                        c1 = min(c0 + CMAX, K)
                        nc.gpsimd.dma_gather(
                            out_ap=adt[:, c0:c1, :],
                            in_ap=adpad[:],
                            idxs_ap=ai_t[:, b * (NSLOT // 16) + c0 * 8:b * (NSLOT // 16) + c1 * 8],
                            num_idxs=(c1 - c0) * P,
                            num_idxs_reg=(c1 - c0) * P,
                            elem_size=64,
                        )
                nc.scalar.copy(out=adall[:, b, :, :], in_=adt[:, :, 0:4])
            for b in range(_n_iter):
                zs = ep.tile([P, K, 64], f32, tag="zs")
                if True:
                    for c0 in range(0, KLO, CMAX):
                        c1 = min(c0 + CMAX, KLO)
                        nc.gpsimd.dma_gather(
                            out_ap=zs[:, c0:c1, :],
                            in_ap=zaug_lo,
                            idxs_ap=gl_t[:, b * (NLO // 16) + c0 * 8:b * (NLO // 16) + c1 * 8],
                            num_idxs=(c1 - c0) * P,
                            num_idxs_reg=(c1 - c0) * P,
                            elem_size=64,
                        )
                if True:
                    for c0 in range(0, KHI, CMAX):
                        c1 = min(c0 + CMAX, KHI)
                        nc.gpsimd.dma_gather(
                            out_ap=zs[:, KLO + c0:KLO + c1, :],
                            in_ap=zaug_hi,
                            idxs_ap=gh_t[:, b * (NHI // 16) + c0 * 8:b * (NHI // 16) + c1 * 8],
                            num_idxs=(c1 - c0) * P,
                            num_idxs_reg=(c1 - c0) * P,
                            elem_size=64,
                        )

                # e = as + ad ; exp(leaky(e)) = max(exp(e), exp(0.2*e))
                et = ep.tile([P, K, 4], f32, tag="et")
                nc.vector.tensor_add(out=et[:], in0=zs[:, :, 32:36], in1=adall[:, b, :, :])
                ex1 = ep.tile([P, K, 4], f32, tag="ex1")
                nc.scalar.activation(ex1[:], et[:], mybir.ActivationFunctionType.Exp)
                ext = ep.tile([P, K, 4], f32, tag="ext")
                nc.scalar.activation(ext[:], et[:], mybir.ActivationFunctionType.Exp, scale=ALPHA)
                nc.vector.tensor_tensor(out=ext[:], in0=ext[:], in1=ex1[:], op=mybir.AluOpType.max)
                # rhs = [ex * zs | ex] (converts z bf16 -> f32 in the mul)
                rhs_t = ep.tile([P, K, 68], f32, tag="rhs")
                exb = ext[:].unsqueeze(3).broadcast_to([P, K, 4, 16])
                zsb = zs[:].bitcast(mybir.dt.bfloat16).rearrange(
                    "p k (h f) -> p k h f", h=8)[:, :, 0:4, :]
                rhs4 = rhs_t[:, :, 0:64].rearrange("p k (h f) -> p k h f", h=4)
                nc.vector.tensor_tensor(out=rhs4, in0=zsb, in1=exb, op=mybir.AluOpType.mult)
                nc.scalar.copy(out=rhs_t[:, :, 64:68], in_=ext[:])
                # accumulate [num|denom] over chunks
                acc = accp.tile([P, 68], f32, tag="acc")
                for c in range(K):
                    sel = selp.tile([P, P], f32, tag="sel")
                    nc.vector.tensor_scalar(
                        out=sel[:],
                        in0=iota_t[:],
                        scalar1=dstv_t[:, b * K + c:b * K + c + 1],
                        scalar2=None,
                        op0=mybir.AluOpType.is_equal,
                    )
                    nc.tensor.matmul(
                        out=acc[:],
                        lhsT=sel[:],
                        rhs=rhs_t[:, c, :],
                        start=(c == 0),
                        stop=(c == K - 1),
                    )
                # normalize + store
                dmx = fp.tile([P, 4], f32, tag="dmx")
                nc.vector.tensor_scalar_max(dmx[:], acc[:, 64:68], 1e-12)
                rec = fp.tile([P, 4], f32, tag="rec")
                nc.vector.reciprocal(rec[:], dmx[:])
                ot = fp.tile([P, HF], f32, tag="ot")
                rb = rec[:].unsqueeze(2).broadcast_to([P, 4, 16])
                o4 = ot[:].rearrange("p (h f) -> p h f", h=4)
                nc.vector.tensor_tensor(out=o4, in0=acc[:, 0:64].rearrange("p (h f) -> p h f", h=4), in1=rb, op=mybir.AluOpType.mult)
                o3 = outO[:].rearrange("(s p) e -> p s e", p=P)
                nc.sync.dma_start(out=o3[:, b, :], in_=ot[:])

    nc.finalize()
    return nc


def kernel(h, src, dst, W, A):
    from concourse.bass_utils import run_bass_kernel_spmd

    in_maps, KLO, KHI, unpack_maps = _build_host_data(h, src, dst, W, A)
    nc = _build_program(KLO, KHI)
    res = run_bass_kernel_spmd(nc, in_maps, core_ids=list(range(N_CORES)))
    out = np.empty((N_NODES, HF), dtype=np.float32)
    for k in range(N_CORES):
        out[k * NODES_PER_CORE:(k + 1) * NODES_PER_CORE] = \
            res.results[k]["outO"][unpack_maps[k]]
    return out



# revision 11
# speedup vs baseline: 1.4243x; 1.4243x over previous
"""GAT layer (nn_GATLayer) on 8 Trainium2 NeuronCores via Bass/Tile.

Strategy (dst-partitioned; degree-aligned slots, softmax fully local per core):
  - Core k owns dst nodes [k*6250, (k+1)*6250). Each owned node is pinned to a
    (block, partition) slot; ALL of its in-edges occupy that partition across
    the block's chunks. Segment-softmax then needs no scatter at all: the
    per-node sums are free-axis reductions, and the dst attention term ad is a
    per-partition scalar broadcast.
  - Phase A (replicated): zaug[row] = [z bf16 x64 | as f32 x4 | pad] (256B rows)
    for ALL nodes via one matmul with folded weights [W.T | W.T@A1blk].
    Rows are partition-major (row = p*391 + s) so stores are contiguous.
    Two reserved rows (0 and 50047) get as = -60 patched in: pad slots gather
    them and contribute exp(leaky(-60+ad)) ~ 1e-5 to denom and 0 to num (z=0).
  - Phase A0: ad = hT_own @ (W.T@A2blk) for own nodes in (block, pos) order;
    stays SBUF-resident [128, 49, 4].
  - Edge phase per block: dma_gather of zaug rows by src (two overlapping
    int16 views: A = rows [0, 32768), B = rows [17280, 50048); per-node lo/hi
    edge split chosen on host), e = as + ad, ex = max(exp(e), exp(0.2e)),
    rhs = [ex*zs | ex], then one reduce_sum over chunks -> [num | denom];
    out = num * recip(denom), accumulated in SBUF, single store at the end.
  - Gather calls span several blocks (SWDGE ring enlarged to 4096 descs) to
    amortize the per-call desc-gen overhead on Pool.

All index/layout prep (row permutation placing high-out-degree nodes in the
overlapping view region, per-core 2D block packing, per-block KLO/KHI chunk
budgets uniformized across cores so one program serves all 8) is done on host.
"""

import numpy as np

N_NODES = 50000
N_EDGES = 800000
IN_FEATS = 128
OUT_FEATS = 16
NUM_HEADS = 4
ALPHA = 0.2
HF = NUM_HEADS * OUT_FEATS  # 64

N_CORES = 8
P = 128
NODES_PER_CORE = N_NODES // N_CORES     # 6250
BLOCKS = 49                              # ceil(6250/128)
NODE_PAD = BLOCKS * P                    # 6272
NCHUNK = 391                             # zaug chunks; 128*391 = 50048 rows
N_NODES_PAD = P * NCHUNK                 # 50048
VIEW = 32768                             # int16 gather view size
HIB = N_NODES_PAD - VIEW                 # 17280 = base of view B
PAD_A_ROW = 0                            # reserved pad row in view A (p0, s0)
PAD_B_ROW = 96 * NCHUNK                  # reserved pad row in view B (p96, s0)
DMA_SCRATCH = 16384                      # SWDGE ring: 1024 descriptors
CALL_CHUNKS = 8                          # max chunks (128 idx each) per gather
TILE_CHUNKS = 24                         # max chunks per stream per zs tile


def _wrap16(vals):
    # gather idx layout: stream position i -> idx tile [16, n/16] at
    # [i%16, i//16]; rows replicated to 128 partitions.
    n = vals.shape[-1]
    w = vals.reshape(n // 16, 16).T                    # [16, n/16]
    return np.tile(w, (8, 1))                          # [128, n/16]


def _plan_groups(klos, khis):
    """Greedy grouping of consecutive blocks into zs-tile groups with
    sum(KLO) <= TILE_CHUNKS and sum(KHI) <= TILE_CHUNKS (single blocks may
    exceed the cap; their gathers are split into CALL_CHUNKS-sized calls)."""
    groups = []
    cur = []
    sa = sb = 0
    for b in range(len(klos)):
        ka, kb = klos[b], khis[b]
        if cur and (sa + ka > TILE_CHUNKS or sb + kb > TILE_CHUNKS):
            groups.append(cur)
            cur, sa, sb = [], 0, 0
        cur.append(b)
        sa += ka
        sb += kb
    if cur:
        groups.append(cur)
    return groups


def _build_host_data(h, src, dst, W, A):
    import ml_dtypes

    src = np.asarray(src)
    dst = np.asarray(dst)
    W = np.asarray(W, dtype=np.float32)
    A = np.asarray(A, dtype=np.float32)
    h = np.asarray(h, dtype=np.float32)

    # folded weights
    A1blk = np.zeros((HF, NUM_HEADS), dtype=np.float32)
    A2blk = np.zeros((HF, NUM_HEADS), dtype=np.float32)
    for hd in range(NUM_HEADS):
        A1blk[hd * OUT_FEATS:(hd + 1) * OUT_FEATS, hd] = A[hd, :OUT_FEATS]
        A2blk[hd * OUT_FEATS:(hd + 1) * OUT_FEATS, hd] = A[hd, OUT_FEATS:]
    WT = np.ascontiguousarray(W.T)                                  # [128, 64]
    wcat = np.concatenate([WT, WT @ A1blk], axis=1).astype(ml_dtypes.bfloat16)
    wad = (WT @ A2blk).astype(ml_dtypes.bfloat16)                   # [128, 4]

    # global row permutation: high-out-degree nodes -> overlap rows
    # [HIB, VIEW); rows 0 and 50047 reserved for pad targets.
    outdeg = np.bincount(src, minlength=N_NODES)
    nodes_by_heat = np.argsort(-outdeg, kind="stable")
    ov_rows = np.arange(HIB, VIEW)
    rest_hi = np.arange(VIEW, N_NODES_PAD)
    rest = np.concatenate(
        [np.arange(1, HIB), rest_hi[rest_hi != PAD_B_ROW]])
    perm = np.empty(N_NODES, dtype=np.int64)
    perm[nodes_by_heat[:len(ov_rows)]] = ov_rows
    perm[nodes_by_heat[len(ov_rows):]] = rest[:N_NODES - len(ov_rows)]

    # hT column for row r: phase A chunk s partition p -> row p*391 + s,
    # processed from hT col s*128 + p.
    hT = np.zeros((P, N_NODES_PAD), dtype=ml_dtypes.bfloat16)
    cols = (perm % NCHUNK) * P + perm // NCHUNK
    hT[:, cols] = h.T.astype(ml_dtypes.bfloat16)

    # per-core edge prep
    order = np.argsort(dst, kind="stable")
    dst_s = dst[order]
    rows_s = perm[src[order]]
    core_begin = np.searchsorted(
        dst_s, np.arange(0, N_NODES + 1, NODES_PER_CORE))

    cores = []
    for k in range(N_CORES):
        lo_e, hi_e = core_begin[k], core_begin[k + 1]
        cd = dst_s[lo_e:hi_e] - k * NODES_PER_CORE
        rw = rows_s[lo_e:hi_e]
        is_a = rw < HIB                      # A-only
        is_b = rw >= VIEW                    # B-only
        is_f = ~is_a & ~is_b                 # flexible
        a = np.bincount(cd[is_a], minlength=NODES_PER_CORE)
        c = np.bincount(cd[is_b], minlength=NODES_PER_CORE)
        f = np.bincount(cd[is_f], minlength=NODES_PER_CORE)
        T = a + c + f
        node_order = np.lexsort((a, -T // 2))
        # node -> (block, pos)
        node_block = np.empty(NODES_PER_CORE, dtype=np.int64)
        node_pos = np.empty(NODES_PER_CORE, dtype=np.int64)
        node_block[node_order] = np.arange(NODES_PER_CORE) // P
        node_pos[node_order] = np.arange(NODES_PER_CORE) % P
        # per-block optimal (KLO, KHI)
        klo = np.zeros(BLOCKS, dtype=np.int64)
        khi = np.zeros(BLOCKS, dtype=np.int64)
        for b in range(BLOCKS):
            blk = node_order[b * P:(b + 1) * P]
            ab, cb_, fb, Tb = a[blk], c[blk], f[blk], T[blk]
            best = None
            for KLO in range(int(ab.max()), int(Tb.max()) + 1):
                KHI = int(np.maximum(cb_, Tb - np.minimum(KLO, ab + fb)).max())
                if best is None or KLO + KHI < best[0]:
                    best = (KLO + KHI, KLO, KHI)
                if KHI == int(cb_.max()):
                    break
            klo[b], khi[b] = best[1], best[2]
        cores.append(dict(cd=cd, rw=rw, a=a, c=c, f=f, T=T,
                          node_block=node_block, node_pos=node_pos,
                          klo=klo, khi=khi))

    # uniform per-block chunk budgets across cores (one program, 8 cores)
    KLOs = np.max([co["klo"] for co in cores], axis=0)
    KHIs = np.max([co["khi"] for co in cores], axis=0)
    groups = _plan_groups(KLOs, KHIs)
    LA = int(KLOs.sum()) * P
    LB = int(KHIs.sum()) * P
    offA = np.concatenate([[0], np.cumsum(KLOs)])    # chunk offsets per block
    offB = np.concatenate([[0], np.cumsum(KHIs)])

    in_maps = []
    unpack_maps = []
    for k in range(N_CORES):
        co = cores[k]
        cd, rw = co["cd"], co["rw"]
        a, f, T = co["a"], co["f"], co["T"]
        node_block, node_pos = co["node_block"], co["node_pos"]
        # per-node lo count: L = max(a, T - KHI_block)
        KHI_n = KHIs[node_block]
        L = np.maximum(a, T - KHI_n)

        # sort edges by (node, flexibility-class) so each node's edge list is
        # [A-only..., flex..., B-only...]; first L edges -> stream A.
        cls = np.where(rw < HIB, 0, np.where(rw < VIEW, 1, 2))
        eo = np.lexsort((cls, cd))
        cd_o, rw_o = cd[eo], rw[eo]
        starts = np.searchsorted(cd_o, np.arange(NODES_PER_CORE + 1))
        rank = np.arange(len(cd_o)) - starts[cd_o]          # rank within node
        to_a = rank < L[cd_o]

        gA = np.full((LA // P, P), PAD_A_ROW, dtype=np.int16)
        gB = np.full((LB // P, P), PAD_B_ROW - HIB, dtype=np.int16)
        # slot chunk = offX[block] + rank (A) or rank - L (B)
        blk_e = node_block[cd_o]
        pos_e = node_pos[cd_o]
        ca = offA[blk_e] + rank
        cb_ = offB[blk_e] + rank - L[cd_o]
        gA[ca[to_a], pos_e[to_a]] = rw_o[to_a].astype(np.int16)
        gB[cb_[~to_a], pos_e[~to_a]] = (rw_o[~to_a] - HIB).astype(np.int16)

        # wrap16 per call group
        gAw, gBw = [], []
        for g in groups:
            b0, b1 = g[0], g[-1] + 1
            gAw.append(_wrap16(gA[offA[b0]:offA[b1]].reshape(-1)))
            gBw.append(_wrap16(gB[offB[b0]:offB[b1]].reshape(-1)))
        gAw = np.ascontiguousarray(np.concatenate(gAw, axis=1))
        gBw = np.ascontiguousarray(np.concatenate(gBw, axis=1))

        # hT_own: col b*128 + pos = h[node]
        hT_own = np.zeros((P, NODE_PAD), dtype=ml_dtypes.bfloat16)
        own = np.arange(k * NODES_PER_CORE, (k + 1) * NODES_PER_CORE)
        hT_own[:, node_block * P + node_pos] = h[own].T.astype(
            ml_dtypes.bfloat16)

        in_maps.append({
            "hT": hT,
            "hT_own": hT_own,
            "wcat": np.ascontiguousarray(wcat),
            "wad": np.ascontiguousarray(wad),
            "gidxA": gAw,
            "gidxB": gBw,
        })
        # outO row for node (block, pos) = pos*BLOCKS + block
        unpack_maps.append(node_pos * BLOCKS + node_block)

    return in_maps, (KLOs, KHIs, groups), unpack_maps


def _build_program(plan):
    import concourse.bacc as bacc
    import concourse.tile as tile
    import concourse.mybir as mybir

    KLOs, KHIs, groups = plan
    LA = int(KLOs.sum()) * P
    LB = int(KHIs.sum()) * P
    f32 = mybir.dt.float32
    bf16 = mybir.dt.bfloat16
    i16 = mybir.dt.int16

    import os as _os
    _simclean = _os.environ.get("SIM_CLEAN", "0") == "1"
    nc = bacc.Bacc("TRN2", target_bir_lowering=False, debug=False,
                   dynamic_dma_scratch_size=DMA_SCRATCH)

    hT = nc.dram_tensor("hT", [P, N_NODES_PAD], bf16, kind="ExternalInput")
    hT_own = nc.dram_tensor("hT_own", [P, NODE_PAD], bf16, kind="ExternalInput")
    wcat_d = nc.dram_tensor("wcat", [P, 68], bf16, kind="ExternalInput")
    wad_d = nc.dram_tensor("wad", [P, 4], bf16, kind="ExternalInput")
    gidxA = nc.dram_tensor("gidxA", [P, LA // 16], i16, kind="ExternalInput")
    gidxB = nc.dram_tensor("gidxB", [P, LB // 16], i16, kind="ExternalInput")

    zaug = nc.dram_tensor("zaug", [N_NODES_PAD, 64], f32)
    outO = nc.dram_tensor("outO", [NODE_PAD, HF], f32, kind="ExternalOutput")

    SC = 4                        # chunks per PSUM tile
    SC2 = 8                       # chunks per load/store superchunk

    with tile.TileContext(nc) as tc:
        with (
            tc.tile_pool(name="const", bufs=1) as cpool,
            tc.tile_pool(name="pa", bufs=8) as pa,
            tc.tile_pool(name="papsum", bufs=4, space="PSUM") as papsum,
            tc.tile_pool(name="adpsum", bufs=1, space="PSUM") as adpsum,
            tc.tile_pool(name="epA", bufs=2) as epA,
            tc.tile_pool(name="epB", bufs=2) as epB,
            tc.tile_pool(name="ep", bufs=3) as ep,
            tc.tile_pool(name="fp", bufs=3) as fp,
        ):
            wcat_t = cpool.tile([P, 68], bf16)
            nc.sync.dma_start(out=wcat_t[:], in_=wcat_d[:])
            wad_t = cpool.tile([P, 4], bf16)
            nc.sync.dma_start(out=wad_t[:], in_=wad_d[:])
            ho_t = cpool.tile([P, NODE_PAD], bf16)
            nc.sync.dma_start(out=ho_t[:], in_=hT_own[:])
            gA_t = cpool.tile([P, LA // 16], i16)
            nc.sync.dma_start(out=gA_t[:], in_=gidxA[:])
            gB_t = cpool.tile([P, LB // 16], i16)
            nc.sync.dma_start(out=gB_t[:], in_=gidxB[:])

            # ---------------- Phase A0: ad for own nodes (SBUF resident) ----
            adp = adpsum.tile([P, BLOCKS, 4], f32)
            for b in range(BLOCKS):
                nc.tensor.matmul(
                    out=adp[:, b, :],
                    lhsT=ho_t[:, b * P:(b + 1) * P],
                    rhs=wad_t[:],
                    start=True, stop=True,
                )
            adall = cpool.tile([P, BLOCKS, 4], f32)
            nc.scalar.copy(out=adall[:], in_=adp[:])

            # ---------------- Phase A: zaug for all nodes -------------------
            n_sc = NCHUNK // SC2
            all_scs = [(s * SC2, SC2) for s in range(n_sc)]
            if NCHUNK % SC2:
                all_scs.append((n_sc * SC2, NCHUNK % SC2))
            z3 = zaug[:].rearrange("(p s) e -> p s e", s=NCHUNK)
            for s0, nsub in all_scs:
                hsl = pa.tile([P, SC2 * P], bf16, tag="hsl")
                nc.sync.dma_start(
                    out=hsl[:, :nsub * P],
                    in_=hT[:, s0 * P:(s0 + nsub) * P])
                zst = pa.tile([P, SC2, 64], f32, tag="zst")
                if _simclean:
                    nc.scalar.memzero(zst[:])
                zbf = zst[:].bitcast(mybir.dt.bfloat16)
                for g0 in range(0, nsub, SC):
                    g1 = min(g0 + SC, nsub)
                    zp = papsum.tile([P, SC, 68], f32, tag="zp")
                    for j in range(g0, g1):
                        nc.tensor.matmul(
                            out=zp[:, j - g0, :],
                            lhsT=hsl[:, j * P:(j + 1) * P],
                            rhs=wcat_t[:],
                            start=True, stop=True,
                        )
                    nc.scalar.copy(out=zbf[:, g0:g1, 0:64],
                                   in_=zp[:, :g1 - g0, 0:64])
                    nc.scalar.copy(out=zst[:, g0:g1, 32:36],
                                   in_=zp[:, :g1 - g0, 64:68])
                # overwrite as = -60 on the reserved pad rows (p, s) =
                # (0, 0) and (96, 0) before the store
                if s0 == 0:
                    nc.vector.memset(zst[0:1, 0, 32:36], -60.0)
                    nc.vector.memset(zst[96:97, 0, 32:36], -60.0)
                nc.sync.dma_start(out=z3[:, s0:s0 + nsub, :],
                                  in_=zst[:, :nsub, :])

            # ---------------- Edge phase ------------------------------------
            viewA = zaug[0:VIEW, :]
            viewB = zaug[HIB:N_NODES_PAD, :]
            outS = cpool.tile([P, BLOCKS, HF], f32)
            offA = np.concatenate([[0], np.cumsum(KLOs)])
            offB = np.concatenate([[0], np.cumsum(KHIs)])
            o16A = o16B = 0
            for g in groups:
                b0, b1 = g[0], g[-1] + 1
                ga_ch = int(offA[b1] - offA[b0])
                gb_ch = int(offB[b1] - offB[b0])
                zsA = epA.tile([P, ga_ch, 64], f32, tag="zsA")
                for c0 in range(0, ga_ch, CALL_CHUNKS):
                    c1 = min(c0 + CALL_CHUNKS, ga_ch)
                    nc.gpsimd.dma_gather(
                        out_ap=zsA[:, c0:c1, :],
                        in_ap=viewA,
                        idxs_ap=gA_t[:, o16A + c0 * 8:o16A + c1 * 8],
                        num_idxs=(c1 - c0) * P,
                        num_idxs_reg=(c1 - c0) * P,
                        elem_size=64,
                    )
                zsB = epB.tile([P, gb_ch, 64], f32, tag="zsB")
                for c0 in range(0, gb_ch, CALL_CHUNKS):
                    c1 = min(c0 + CALL_CHUNKS, gb_ch)
                    nc.gpsimd.dma_gather(
                        out_ap=zsB[:, c0:c1, :],
                        in_ap=viewB,
                        idxs_ap=gB_t[:, o16B + c0 * 8:o16B + c1 * 8],
                        num_idxs=(c1 - c0) * P,
                        num_idxs_reg=(c1 - c0) * P,
                        elem_size=64,
                    )
                o16A += ga_ch * 8
                o16B += gb_ch * 8
                for b in g:
                    KA, KB = int(KLOs[b]), int(KHIs[b])
                    K = KA + KB
                    ca = int(offA[b] - offA[b0])
                    cb_ = int(offB[b] - offB[b0])
                    adb = adall[:, b, :].unsqueeze(1)
                    et = ep.tile([P, K, 4], f32, tag="et")
                    nc.vector.tensor_add(
                        out=et[:, 0:KA, :],
                        in0=zsA[:, ca:ca + KA, 32:36],
                        in1=adb.broadcast_to([P, KA, 4]))
                    nc.vector.tensor_add(
                        out=et[:, KA:K, :],
                        in0=zsB[:, cb_:cb_ + KB, 32:36],
                        in1=adb.broadcast_to([P, KB, 4]))
                    ex1 = ep.tile([P, K, 4], f32, tag="ex1")
                    nc.scalar.activation(ex1[:], et[:],
                                         mybir.ActivationFunctionType.Exp)
                    ext = ep.tile([P, K, 4], f32, tag="ext")
                    nc.scalar.activation(ext[:], et[:],
                                         mybir.ActivationFunctionType.Exp,
                                         scale=ALPHA)
                    nc.vector.tensor_tensor(out=ext[:], in0=ext[:], in1=ex1[:],
                                            op=mybir.AluOpType.max)
                    # rhs = [ex * zs | ex]
                    rhs_t = ep.tile([P, K, 68], f32, tag="rhs")
                    exb = ext[:].unsqueeze(3)
                    zsbA = zsA[:, ca:ca + KA, 0:32].bitcast(bf16).rearrange(
                        "p k (h f) -> p k h f", h=4)
                    nc.vector.tensor_tensor(
                        out=rhs_t[:, 0:KA, 0:64].rearrange(
                            "p k (h f) -> p k h f", h=4),
                        in0=zsbA,
                        in1=exb[:, 0:KA, :, :].broadcast_to([P, KA, 4, 16]),
                        op=mybir.AluOpType.mult)
                    zsbB = zsB[:, cb_:cb_ + KB, 0:32].bitcast(bf16).rearrange(
                        "p k (h f) -> p k h f", h=4)
                    nc.vector.tensor_tensor(
                        out=rhs_t[:, KA:K, 0:64].rearrange(
                            "p k (h f) -> p k h f", h=4),
                        in0=zsbB,
                        in1=exb[:, KA:K, :, :].broadcast_to([P, KB, 4, 16]),
                        op=mybir.AluOpType.mult)
                    nc.scalar.copy(out=rhs_t[:, :, 64:68], in_=ext[:])
                    # [num | denom]: pairwise tree-sum over chunks (all
                    # operands contiguous, f32 accumulation)
                    n = K
                    while n > 1:
                        hh = n // 2
                        nc.vector.tensor_add(
                            out=rhs_t[:, 0:hh, :],
                            in0=rhs_t[:, 0:hh, :],
                            in1=rhs_t[:, n - hh:n, :])
                        n -= hh
                    red = rhs_t[:, 0, :]
                    rec = fp.tile([P, 4], f32, tag="rec")
                    nc.vector.reciprocal(rec[:], red[:, 64:68])
                    nc.vector.tensor_tensor(
                        out=outS[:, b, :].rearrange("p (h f) -> p h f", h=4),
                        in0=red[:, 0:64].rearrange("p (h f) -> p h f", h=4),
                        in1=rec[:].unsqueeze(2).broadcast_to([P, 4, 16]),
                        op=mybir.AluOpType.mult)

            o3 = outO[:].rearrange("(p s) e -> p s e", s=BLOCKS)
            nc.sync.dma_start(out=o3[:], in_=outS[:])

    nc.finalize()
    return nc


def kernel(h, src, dst, W, A):
    from concourse.bass_utils import run_bass_kernel_spmd

    in_maps, plan, unpack_maps = _build_host_data(h, src, dst, W, A)
    nc = _build_program(plan)
    res = run_bass_kernel_spmd(nc, in_maps, core_ids=list(range(N_CORES)))
    out = np.empty((N_NODES, HF), dtype=np.float32)
    for k in range(N_CORES):
        out[k * NODES_PER_CORE:(k + 1) * NODES_PER_CORE] = \
            res.results[k]["outO"][unpack_maps[k]]
    return out
